# revision 9
# baseline (speedup 1.0000x reference)
"""NetVLAD pooling kernel for 8 Trainium2 NeuronCores.

Computes, for x:(64,1024,512), clusters:(512,64), clusters2:(1,512,64),
gamma/beta:(64,):
    a   = BatchNorm(x.reshape(-1,512) @ clusters)   (training-mode batch stats)
    s   = softmax(a, axis=-1).reshape(64,1024,64)
    v   = einsum('bnk,bnd->bdk', s, x) - s.sum(1)[:,None,:]*clusters2
    out = L2-normalize(v.reshape(64, 512*64), axis=1)

Sharding: data-parallel over batch (8 batches/core); BatchNorm batch stats
are combined exactly with a tiny (64x2 fp32) AllReduce across the 8 cores.
"""

import math
import os
import sys
from contextlib import ExitStack

import numpy as np

for _p in ("/opt/trn_rl_repo", "/root/.axon_site/_ro/trn_rl_repo"):
    if os.path.isdir(_p) and _p not in sys.path:
        sys.path.insert(0, _p)

import concourse.bass as bass
import concourse.tile as tile
from concourse import bacc, mybir
from concourse import bass_utils
from concourse.masks import make_identity

F32 = mybir.dt.float32

# Problem shape (hardcoded per spec)
B, N, D, K = 64, 1024, 512, 64
BN_EPS = 1e-5
L2_EPS = 1e-8
N_CORES = 8
B_LOC = B // N_CORES            # 8 batches per core
R = B_LOC * N                   # 8192 rows per core
T = R // 128                    # 64 row-tiles of 128
DCH = D // 128                  # 4 chunks of the feature dim
G = R // 512                    # 16 row-groups of 512
GP = G // 2                     # 8 group pairs (packed into 128 psum partitions)
N_TOTAL = B * N                 # BN batch size (65536)

# dtype knobs for the two big matmul families (accuracy/perf tradeoff)
MM1_DT = getattr(mybir.dt, os.environ.get("NV_MM1_DT", "float32"))
MM2_DT = getattr(mybir.dt, os.environ.get("NV_MM2_DT", "float32"))
TR_DT = getattr(mybir.dt, os.environ.get("NV_TR_DT", "float32"))

_cached = {}


def _mm_ap(ap, dt):
    return ap if dt == F32 else ap.bitcast(dt)


def build_kernel():
    """Build + compile the per-core Bass module (same program on all 8 cores)."""
    nc = bacc.Bacc("TRN2", target_bir_lowering=False, debug=False,
                   num_devices=N_CORES)

    x_d = nc.dram_tensor("x", [R, D], F32, kind="ExternalInput")
    cl_d = nc.dram_tensor("clusters", [D, K], F32, kind="ExternalInput")
    c2_d = nc.dram_tensor("clusters2", [D, K], F32, kind="ExternalInput")
    ga_d = nc.dram_tensor("gamma", [K, 1], F32, kind="ExternalInput")
    be_d = nc.dram_tensor("beta", [K, 1], F32, kind="ExternalInput")
    out_d = nc.dram_tensor("out", [B_LOC, D * K], F32, kind="ExternalOutput")

    with tile.TileContext(nc) as tc, ExitStack() as ctx:
        singles = ctx.enter_context(tc.tile_pool(name="singles", bufs=1))
        xpool = ctx.enter_context(tc.tile_pool(name="xnat", bufs=1))
        apool = ctx.enter_context(tc.tile_pool(name="aT", bufs=1))
        spool = ctx.enter_context(tc.tile_pool(name="soft", bufs=1))
        xtg_pool = ctx.enter_context(tc.tile_pool(name="xtg", bufs=2))
        work = ctx.enter_context(tc.tile_pool(name="work", bufs=2))
        # PSUM pools: exactly 8 banks total
        tpsum = ctx.enter_context(tc.tile_pool(name="tpsum", bufs=2, space="PSUM"))
        psA = ctx.enter_context(tc.tile_pool(name="psA", bufs=2, space="PSUM"))
        psV = ctx.enter_context(tc.tile_pool(name="psV", bufs=2, space="PSUM"))
        psS = ctx.enter_context(tc.tile_pool(name="psS", bufs=2, space="PSUM"))
        dram = ctx.enter_context(tc.tile_pool(name="dram", bufs=1, space="DRAM"))

        # ---- constants ----------------------------------------------------
        identity = singles.tile([128, 128], F32)
        make_identity(nc, identity[:])
        # I64 living on partitions 64..127 (for transposing the packed upper half)
        ident_hi = singles.tile([128, K], F32)
        nc.gpsimd.memset(ident_hi[:], 0.0)
        nc.gpsimd.affine_select(out=ident_hi[:], in_=ident_hi[:],
                                compare_op=mybir.AluOpType.not_equal, fill=1.0,
                                base=-64, pattern=[[-1, K]], channel_multiplier=1)
        ones_col = singles.tile([128, 1], F32)
        nc.vector.memset(ones_col[:], 1.0)
        ones_row = singles.tile([1, K], F32)
        nc.vector.memset(ones_row[:], 1.0)
        eps_t = singles.tile([K, 1], F32)
        nc.vector.memset(eps_t[:], BN_EPS)
        # stackselT[q, f] = 1 iff f == q or f == q + 64  (replicate [64]->[128])
        stackselT = singles.tile([K, 128], F32)
        nc.gpsimd.memset(stackselT[:], 0.0)
        nc.gpsimd.affine_select(out=stackselT[:], in_=stackselT[:],
                                compare_op=mybir.AluOpType.not_equal, fill=1.0,
                                base=0, pattern=[[-1, 128]], channel_multiplier=1)
        nc.gpsimd.affine_select(out=stackselT[:], in_=stackselT[:],
                                compare_op=mybir.AluOpType.not_equal, fill=1.0,
                                base=64, pattern=[[-1, 128]], channel_multiplier=1)

        clusters_sb = singles.tile([128, DCH, K], F32)
        nc.sync.dma_start(clusters_sb[:], cl_d.ap().rearrange("(c p) k -> p c k", p=128))
        c2nat = singles.tile([128, DCH, K], F32)
        nc.sync.dma_start(c2nat[:], c2_d.ap().rearrange("(c p) k -> p c k", p=128))
        gamma_sb = singles.tile([K, 1], F32)
        nc.sync.dma_start(gamma_sb[:], ga_d.ap())
        beta_sb = singles.tile([K, 1], F32)
        nc.sync.dma_start(beta_sb[:], be_d.ap())

        # clusters2^T : [K, D]
        c2T = singles.tile([K, D], F32)
        for c in range(DCH):
            tp = tpsum.tile([K, 128], F32, tag="tp")
            nc.tensor.transpose(tp[:], c2nat[:, c, :], identity[:])
            nc.scalar.copy(c2T[:, 128 * c:128 * (c + 1)], tp[:])

        # ---- load x -------------------------------------------------------
        xnat = xpool.tile([128, T, D], F32)
        x_view = x_d.ap().rearrange("(t p) d -> p t d", p=128)
        for b in range(B_LOC):
            nc.sync.dma_start(xnat[:, 8 * b:8 * (b + 1), :],
                              x_view[:, 8 * b:8 * (b + 1), :])

        # ---- pass 1: assignment^T = clusters^T @ x^T  (packed 2 groups) ---
        # aT[128, T*64]: partition p<64 holds k=p for even groups, p>=64 holds
        # k=p-64 for odd groups; free = 512*(g//2) + row-in-group.
        aT = apool.tile([128, GP * 512], F32)
        for i in range(GP):
            a_ps = psA.tile([128, 512], F32)
            for h in range(2):
                g = 2 * i + h
                xT_g = xtg_pool.tile([128, DCH, 4, 128], F32, tag="xtg")
                for tin in range(4):
                    t = 4 * g + tin
                    tp = tpsum.tile([128, DCH, 128], F32, tag="tp")
                    for c in range(DCH):
                        nc.tensor.transpose(_mm_ap(tp[:, c, :], TR_DT),
                                            _mm_ap(xnat[:, t, 128 * c:128 * (c + 1)], TR_DT),
                                            _mm_ap(identity[:], TR_DT))
                    eng = nc.vector if (t % 2 == 0) else nc.scalar
                    if eng is nc.vector:
                        eng.tensor_copy(xT_g[:, :, tin, :], tp[:])
                    else:
                        eng.copy(xT_g[:, :, tin, :], tp[:])
                for c in range(DCH):
                    nc.tensor.matmul(
                        a_ps[64 * h:64 * (h + 1), :],
                        _mm_ap(clusters_sb[:, c, :], MM1_DT),
                        _mm_ap(xT_g[:, c].rearrange("p a b -> p (a b)"), MM1_DT),
                        start=(c == 0), stop=(c == DCH - 1))
            nc.vector.tensor_copy(aT[:, 512 * i:512 * (i + 1)], a_ps[:])

        # ---- BN statistics ------------------------------------------------
        stats = work.tile([128, GP, nc.vector.BN_STATS_DIM], F32, tag="stats")
        for i in range(GP):
            nc.vector.bn_stats(stats[:, i, :], aT[:, 512 * i:512 * (i + 1)])
        mv = work.tile([128, 2], F32, tag="mv")       # (mean, var) per partition
        nc.vector.bn_aggr(mv[:], stats[:])
        musq = work.tile([128, 1], F32, tag="musq")
        nc.vector.tensor_mul(musq[:], mv[:, 0:1], mv[:, 0:1])
        nc.vector.tensor_add(mv[:, 1:2], mv[:, 1:2], musq[:])   # E[a^2] per half
        # combine partition halves (same k lives at p and p+64): via tiny DMAs
        mvc = work.tile([K, 2, 2], F32, tag="mvc")
        nc.gpsimd.dma_start(mvc[:, 0, :], mv[0:K, :])
        nc.gpsimd.dma_start(mvc[:, 1, :], mv[K:128, :])
        sums = work.tile([K, 2], F32, tag="sums")     # 2*core-mean of (mu, E2)
        nc.vector.tensor_add(sums[:], mvc[:, 0, :], mvc[:, 1, :])

        # ---- AllReduce of (mu, E2) across the 8 cores ---------------------
        ar_in = dram.tile([K, 2], F32)
        ar_out = dram.tile([K, 2], F32)
        nc.gpsimd.dma_start(ar_in[:], sums[:])
        nc.gpsimd.collective_compute(
            "AllReduce", mybir.AluOpType.add,
            replica_groups=[list(range(N_CORES))],
            ins=[ar_in.opt()], outs=[ar_out.opt()])
        ars = work.tile([K, 2], F32, tag="ars")
        nc.gpsimd.dma_start(ars[:], ar_out[:])

        # ---- BN scale/bias ------------------------------------------------
        mu = work.tile([K, 1], F32, tag="mu")
        nc.vector.tensor_scalar_mul(mu[:], ars[:, 0:1], 1.0 / (2 * N_CORES))
        var = work.tile([K, 1], F32, tag="var")
        nc.vector.tensor_scalar_mul(var[:], ars[:, 1:2], 1.0 / (2 * N_CORES))
        nc.vector.tensor_mul(musq[0:K, :], mu[:], mu[:])
        nc.vector.tensor_sub(var[:], var[:], musq[0:K, :])
        std = work.tile([K, 1], F32, tag="std")
        nc.scalar.activation(std[:], var[:], mybir.ActivationFunctionType.Sqrt,
                             bias=eps_t[:], scale=1.0)
        rstd = work.tile([K, 1], F32, tag="rstd")
        nc.vector.reciprocal(rstd[:], std[:])
        scale64 = work.tile([K, 1], F32, tag="scale64")
        nc.vector.tensor_mul(scale64[:], rstd[:], gamma_sb[:])
        bias64 = work.tile([K, 1], F32, tag="bias64")
        nc.vector.tensor_mul(bias64[:], mu[:], scale64[:])
        nc.vector.tensor_sub(bias64[:], beta_sb[:], bias64[:])
        # replicate [64,1] -> [128,1] so both packed halves get their k's coeff
        sc_ps = psS.tile([128, 1], F32, tag="smallps")
        nc.tensor.matmul(sc_ps[:], stackselT[:], scale64[:], start=True, stop=True)
        scale128 = work.tile([128, 1], F32, tag="scale128")
        nc.scalar.copy(scale128[:], sc_ps[:])
        bi_ps = psS.tile([128, 1], F32, tag="smallps")
        nc.tensor.matmul(bi_ps[:], stackselT[:], bias64[:], start=True, stop=True)
        bias128 = work.tile([128, 1], F32, tag="bias128")
        nc.scalar.copy(bias128[:], bi_ps[:])

        # ---- exp(BN(a)) in place, then transpose to n-major + normalize ---
        nc.scalar.activation(aT[:], aT[:], mybir.ActivationFunctionType.Exp,
                             bias=bias128[:], scale=scale128[:])

        soft = spool.tile([128, T, K], F32)
        zsum = work.tile([128, T], F32, tag="zsum")
        for t in range(T):
            g = t // 4
            h = g % 2
            off = 512 * (g // 2) + 128 * (t % 4)
            sp = tpsum.tile([128, K], F32, tag="tp")
            ident_h = identity[0:K, 0:K] if h == 0 else ident_hi[64:128, :]
            nc.tensor.transpose(sp[:], aT[64 * h:64 * (h + 1), off:off + 128],
                                ident_h)
            nc.scalar.copy(soft[:, t, :], sp[:])
            nc.vector.reduce_sum(zsum[:, t:t + 1], sp[:], axis=mybir.AxisListType.X)
        zr = work.tile([128, T], F32, tag="zr")
        nc.vector.reciprocal(zr[:], zsum[:])
        for t in range(T):
            nc.vector.tensor_scalar_mul(soft[:, t, :], soft[:, t, :], zr[:, t:t + 1])

        # ---- pass 2: vlad^T = soft^T @ x, minus a_sum * clusters2^T -------
        for b in range(B_LOC):
            v_ps = psV.tile([K, 512], F32)
            s_ps = psS.tile([K, 1], F32, tag="smallps")
            for j in range(8):
                t = 8 * b + j
                nc.tensor.matmul(v_ps[:], _mm_ap(soft[:, t, :], MM2_DT),
                                 _mm_ap(xnat[:, t, :], MM2_DT),
                                 start=(j == 0), stop=(j == 7))
                nc.tensor.matmul(s_ps[:], _mm_ap(soft[:, t, :], MM2_DT),
                                 _mm_ap(ones_col[:], MM2_DT),
                                 start=(j == 0), stop=(j == 7))
            asum = work.tile([K, 1], F32, tag="asum")
            nc.scalar.copy(asum[:], s_ps[:])
            corr = work.tile([K, D], F32, tag="corr")
            nc.vector.tensor_scalar_mul(corr[:], c2T[:], asum[:])
            vfin = work.tile([K, D], F32, tag="vfin")
            nc.vector.tensor_sub(vfin[:], v_ps[:], corr[:])
            # L2 norm over all D*K elements of this batch
            scr = work.tile([K, D], F32, tag="scr")
            sq = work.tile([K, 1], F32, tag="sq")
            nc.scalar.activation(scr[:], vfin[:],
                                 mybir.ActivationFunctionType.Square,
                                 accum_out=sq[:])
            n_ps = psS.tile([1, 1], F32, tag="smallps")
            nc.tensor.matmul(n_ps[:], ones_col[0:K, :], sq[:], start=True, stop=True)
            nrm = work.tile([1, 1], F32, tag="nrm")
            nc.scalar.activation(nrm[:], n_ps[:], mybir.ActivationFunctionType.Sqrt)
            nc.vector.tensor_scalar_max(nrm[:], nrm[:], L2_EPS)
            nc.vector.reciprocal(nrm[:], nrm[:])
            b_ps = psS.tile([K, 1], F32, tag="smallps")
            nc.tensor.matmul(b_ps[:], ones_row[:], nrm[:], start=True, stop=True)
            invn = work.tile([K, 1], F32, tag="invn")
            nc.scalar.copy(invn[:], b_ps[:])
            nc.vector.tensor_scalar_mul(vfin[:], vfin[:], invn[:])
            # transpose [K, D] -> [128, DCH, K] and write out
            vout = work.tile([128, DCH, K], F32, tag="vout")
            for c in range(DCH):
                fp = tpsum.tile([128, K], F32, tag="tp")
                nc.tensor.transpose(fp[:], vfin[:, 128 * c:128 * (c + 1)],
                                    identity[0:K, 0:K])
                eng_v = (c % 2 == 0)
                if eng_v:
                    nc.vector.tensor_copy(vout[:, c, :], fp[:])
                else:
                    nc.scalar.copy(vout[:, c, :], fp[:])
            nc.sync.dma_start(
                out_d.ap().rearrange("b (c p k) -> b p c k", p=128, k=K)[b],
                vout[:])

    nc.compile()
    return nc


def _get_nc():
    key = (MM1_DT, MM2_DT, TR_DT)
    if key not in _cached:
        _cached[key] = build_kernel()
    return _cached[key]


def kernel(x=None, clusters=None, clusters2=None, gamma=None, beta=None, **kw):
    # Fall back to the deterministic setup_inputs() values for any input the
    # harness does not supply (they are fixed-seed constants of the problem).
    if clusters is None or clusters2 is None or gamma is None or beta is None:
        import jax
        cpu = jax.devices("cpu")[0]
        with jax.default_device(cpu):
            key = jax.random.key(0)
            k_x, k_c, k_c2 = jax.random.split(key, 3)
            init_sc = 1.0 / math.sqrt(D)
            if clusters is None:
                clusters = np.asarray(init_sc * jax.random.normal(k_c, (D, K)))
            if clusters2 is None:
                clusters2 = np.asarray(init_sc * jax.random.normal(k_c2, (1, D, K)))
            if gamma is None:
                gamma = np.ones((K,), np.float32)
            if beta is None:
                beta = np.zeros((K,), np.float32)
            if x is None:
                x = np.asarray(jax.random.normal(k_x, (B, N, D)))

    x = np.ascontiguousarray(np.asarray(x, dtype=np.float32))
    cl = np.ascontiguousarray(np.asarray(clusters, dtype=np.float32).reshape(D, K))
    c2 = np.ascontiguousarray(np.asarray(clusters2, dtype=np.float32).reshape(D, K))
    ga = np.ascontiguousarray(np.asarray(gamma, dtype=np.float32).reshape(K, 1))
    be = np.ascontiguousarray(np.asarray(beta, dtype=np.float32).reshape(K, 1))

    nc = _get_nc()
    in_maps = []
    for c in range(N_CORES):
        xs = np.ascontiguousarray(
            x[c * B_LOC:(c + 1) * B_LOC].reshape(R, D))
        in_maps.append({"x": xs, "clusters": cl, "clusters2": c2,
                        "gamma": ga, "beta": be})
    res = bass_utils.run_bass_kernel_spmd(
        nc, in_maps, core_ids=list(range(N_CORES)),
        **kw.get("_run_kwargs", {}))
    out = np.concatenate([res.results[c]["out"] for c in range(N_CORES)], axis=0)
    if kw.get("_return_results"):
        return out, res
    return out


# revision 15
# speedup vs baseline: 1.2532x; 1.2532x over previous
"""NetVLAD pooling kernel for 8 Trainium2 NeuronCores.

Computes, for x:(64,1024,512), clusters:(512,64), clusters2:(1,512,64),
gamma/beta:(64,):
    a   = BatchNorm(x.reshape(-1,512) @ clusters)   (training-mode batch stats)
    s   = softmax(a, axis=-1).reshape(64,1024,64)
    v   = einsum('bnk,bnd->bdk', s, x) - s.sum(1)[:,None,:]*clusters2
    out = L2-normalize(v.reshape(64, 512*64), axis=1)

Sharding: data-parallel over batch (8 batches/core); BatchNorm batch stats
are combined exactly with a tiny (64x2 fp32) AllReduce across the 8 cores.
"""

import math
import os
import sys
from contextlib import ExitStack

import numpy as np

for _p in ("/opt/trn_rl_repo", "/root/.axon_site/_ro/trn_rl_repo"):
    if os.path.isdir(_p) and _p not in sys.path:
        sys.path.insert(0, _p)

import concourse.bass as bass
import concourse.tile as tile
from concourse import bacc, mybir
from concourse import bass_utils
from concourse.masks import make_identity

F32 = mybir.dt.float32

# Problem shape (hardcoded per spec)
B, N, D, K = 64, 1024, 512, 64
BN_EPS = 1e-5
L2_EPS = 1e-8
N_CORES = 8
B_LOC = B // N_CORES            # 8 batches per core
R = B_LOC * N                   # 8192 rows per core
T = R // 128                    # 64 row-tiles of 128
DCH = D // 128                  # 4 chunks of the feature dim
G = R // 512                    # 16 row-groups of 512
GP = G // 2                     # 8 group pairs (packed into 128 psum partitions)
N_TOTAL = B * N                 # BN batch size (65536)

# dtype knob for the two big matmul families (accuracy/perf tradeoff):
# float32 (exact, 4 cyc/row) or float32r (reduced-precision streaming, 1 cyc/row)
MM_DT = getattr(mybir.dt, os.environ.get("NV_MM_DT", "float32"))

_cached = {}


def build_kernel():
    """Build + compile the per-core Bass module (same program on all 8 cores)."""
    nc = bacc.Bacc("TRN2", target_bir_lowering=False, debug=False,
                   num_devices=N_CORES)

    x_d = nc.dram_tensor("x", [R, D], F32, kind="ExternalInput")
    cl_d = nc.dram_tensor("clusters", [D, K], F32, kind="ExternalInput")
    c2_d = nc.dram_tensor("clusters2", [D, K], F32, kind="ExternalInput")
    ga_d = nc.dram_tensor("gamma", [K, 1], F32, kind="ExternalInput")
    be_d = nc.dram_tensor("beta", [K, 1], F32, kind="ExternalInput")
    out_d = nc.dram_tensor("out", [B_LOC, D * K], F32, kind="ExternalOutput")

    with tile.TileContext(nc) as tc, ExitStack() as ctx:
        singles = ctx.enter_context(tc.tile_pool(name="singles", bufs=1))
        xpool = ctx.enter_context(tc.tile_pool(name="xnat", bufs=1))
        apool = ctx.enter_context(tc.tile_pool(name="aT", bufs=1))
        spool = ctx.enter_context(tc.tile_pool(name="soft", bufs=1))
        xtg_pool = ctx.enter_context(tc.tile_pool(name="xtg", bufs=2))
        work = ctx.enter_context(tc.tile_pool(name="work", bufs=2))
        # PSUM pools: exactly 8 banks total
        tpsum = ctx.enter_context(tc.tile_pool(name="tpsum", bufs=2, space="PSUM"))
        psA = ctx.enter_context(tc.tile_pool(name="psA", bufs=2, space="PSUM"))
        psV = ctx.enter_context(tc.tile_pool(name="psV", bufs=2, space="PSUM"))
        psS = ctx.enter_context(tc.tile_pool(name="psS", bufs=2, space="PSUM"))
        dram = ctx.enter_context(tc.tile_pool(name="dram", bufs=1, space="DRAM"))

        # ---- constants ----------------------------------------------------
        identity = singles.tile([128, 128], F32)
        make_identity(nc, identity[:])
        # I64 living on partitions 64..127 (for transposing the packed upper half)
        ident_hi = singles.tile([128, K], F32)
        nc.gpsimd.memset(ident_hi[:], 0.0)
        nc.gpsimd.affine_select(out=ident_hi[:], in_=ident_hi[:],
                                compare_op=mybir.AluOpType.not_equal, fill=1.0,
                                base=-64, pattern=[[-1, K]], channel_multiplier=1)
        ones_col = singles.tile([128, 1], F32)
        nc.vector.memset(ones_col[:], 1.0)
        ones_row = singles.tile([1, K], F32)
        nc.vector.memset(ones_row[:], 1.0)
        eps_t = singles.tile([K, 1], F32)
        nc.vector.memset(eps_t[:], BN_EPS)
        # stackselT[q, f] = 1 iff f == q or f == q + 64  (replicate [64]->[128])
        stackselT = singles.tile([K, 128], F32)
        nc.gpsimd.memset(stackselT[:], 0.0)
        nc.gpsimd.affine_select(out=stackselT[:], in_=stackselT[:],
                                compare_op=mybir.AluOpType.not_equal, fill=1.0,
                                base=0, pattern=[[-1, 128]], channel_multiplier=1)
        nc.gpsimd.affine_select(out=stackselT[:], in_=stackselT[:],
                                compare_op=mybir.AluOpType.not_equal, fill=1.0,
                                base=64, pattern=[[-1, 128]], channel_multiplier=1)

        clusters_sb = singles.tile([128, DCH, K], F32)
        nc.sync.dma_start(clusters_sb[:], cl_d.ap().rearrange("(c p) k -> p c k", p=128))
        if MM_DT == F32:
            ident_r = identity
            clusters_r = clusters_sb
        else:
            ident_r = singles.tile([128, 128], MM_DT)
            nc.vector.tensor_copy(ident_r[:], identity[:])
            clusters_r = singles.tile([128, DCH, K], MM_DT)
            nc.vector.tensor_copy(clusters_r[:], clusters_sb[:])
        c2nat = singles.tile([128, DCH, K], F32)
        nc.sync.dma_start(c2nat[:], c2_d.ap().rearrange("(c p) k -> p c k", p=128))
        gamma_sb = singles.tile([K, 1], F32)
        nc.sync.dma_start(gamma_sb[:], ga_d.ap())
        beta_sb = singles.tile([K, 1], F32)
        nc.sync.dma_start(beta_sb[:], be_d.ap())

        # clusters2^T : [K, D]
        c2T = singles.tile([K, D], F32)
        for c in range(DCH):
            tp = tpsum.tile([K, 128], F32, tag="tp")
            nc.tensor.transpose(tp[:], c2nat[:, c, :], identity[:])
            nc.scalar.copy(c2T[:, 128 * c:128 * (c + 1)], tp[:])

        # ---- load x -------------------------------------------------------
        xnat = xpool.tile([128, T, D], MM_DT)
        x_view = x_d.ap().bitcast(MM_DT).rearrange("(t p) d -> p t d", p=128)
        for b in range(B_LOC):
            nc.sync.dma_start(xnat[:, 8 * b:8 * (b + 1), :],
                              x_view[:, 8 * b:8 * (b + 1), :])

        # ---- pass 1: assignment^T = clusters^T @ x^T  (packed 2 groups) ---
        # aT[128, T*64]: partition p<64 holds k=p for even groups, p>=64 holds
        # k=p-64 for odd groups; free = 512*(g//2) + row-in-group.
        aT = apool.tile([128, GP * 512], F32)
        for i in range(GP):
            # fp32r matmuls may not target PSUM at partition base 64, so the
            # two packed halves accumulate in separate banks.
            a_ps_pair = [psA.tile([K, 512], F32, tag="psA", name=f"a_ps_{i}_{h}")
                         for h in range(2)]
            for h in range(2):
                a_ps = a_ps_pair[h]
                g = 2 * i + h
                xT_g = xtg_pool.tile([128, DCH, 4, 128], MM_DT, tag="xtg")
                for tin in range(4):
                    t = 4 * g + tin
                    tp = tpsum.tile([128, DCH, 128], MM_DT, tag="tp")
                    for c in range(DCH):
                        nc.tensor.transpose(tp[:, c, :],
                                            xnat[:, t, 128 * c:128 * (c + 1)],
                                            ident_r[:])
                    eng = nc.vector if (t % 2 == 0) else nc.scalar
                    if eng is nc.vector:
                        eng.tensor_copy(xT_g[:, :, tin, :], tp[:])
                    else:
                        eng.copy(xT_g[:, :, tin, :], tp[:])
                for c in range(DCH):
                    nc.tensor.matmul(
                        a_ps[:],
                        clusters_r[:, c, :],
                        xT_g[:, c].rearrange("p a b -> p (a b)"),
                        start=(c == 0), stop=(c == DCH - 1))
            nc.vector.tensor_copy(aT[0:K, 512 * i:512 * (i + 1)], a_ps_pair[0][:])
            nc.vector.tensor_copy(aT[K:128, 512 * i:512 * (i + 1)], a_ps_pair[1][:])

        # ---- BN statistics ------------------------------------------------
        stats = work.tile([128, GP, nc.vector.BN_STATS_DIM], F32, tag="stats")
        for i in range(GP):
            nc.vector.bn_stats(stats[:, i, :], aT[:, 512 * i:512 * (i + 1)])
        mv = work.tile([128, 2], F32, tag="mv")       # (mean, var) per partition
        nc.vector.bn_aggr(mv[:], stats[:])
        musq = work.tile([128, 1], F32, tag="musq")
        nc.vector.tensor_mul(musq[:], mv[:, 0:1], mv[:, 0:1])
        nc.vector.tensor_add(mv[:, 1:2], mv[:, 1:2], musq[:])   # E[a^2] per half
        # combine partition halves (same k lives at p and p+64): via tiny DMAs
        mvc = work.tile([K, 2, 2], F32, tag="mvc")
        nc.gpsimd.dma_start(mvc[:, 0, :], mv[0:K, :])
        nc.gpsimd.dma_start(mvc[:, 1, :], mv[K:128, :])
        sums = work.tile([K, 2], F32, tag="sums")     # 2*core-mean of (mu, E2)
        nc.vector.tensor_add(sums[:], mvc[:, 0, :], mvc[:, 1, :])

        # ---- AllReduce of (mu, E2) across the 8 cores ---------------------
        ar_in = dram.tile([K, 2], F32)
        ar_out = dram.tile([K, 2], F32)
        nc.gpsimd.dma_start(ar_in[:], sums[:])
        nc.gpsimd.collective_compute(
            "AllReduce", mybir.AluOpType.add,
            replica_groups=[list(range(N_CORES))],
            ins=[ar_in.opt()], outs=[ar_out.opt()])
        ars = work.tile([K, 2], F32, tag="ars")
        nc.gpsimd.dma_start(ars[:], ar_out[:])

        # ---- BN scale/bias ------------------------------------------------
        mu = work.tile([K, 1], F32, tag="mu")
        nc.vector.tensor_scalar_mul(mu[:], ars[:, 0:1], 1.0 / (2 * N_CORES))
        var = work.tile([K, 1], F32, tag="var")
        nc.vector.tensor_scalar_mul(var[:], ars[:, 1:2], 1.0 / (2 * N_CORES))
        nc.vector.tensor_mul(musq[0:K, :], mu[:], mu[:])
        nc.vector.tensor_sub(var[:], var[:], musq[0:K, :])
        std = work.tile([K, 1], F32, tag="std")
        nc.scalar.activation(std[:], var[:], mybir.ActivationFunctionType.Sqrt,
                             bias=eps_t[:], scale=1.0)
        rstd = work.tile([K, 1], F32, tag="rstd")
        nc.vector.reciprocal(rstd[:], std[:])
        scale64 = work.tile([K, 1], F32, tag="scale64")
        nc.vector.tensor_mul(scale64[:], rstd[:], gamma_sb[:])
        bias64 = work.tile([K, 1], F32, tag="bias64")
        nc.vector.tensor_mul(bias64[:], mu[:], scale64[:])
        nc.vector.tensor_sub(bias64[:], beta_sb[:], bias64[:])
        # replicate [64,1] -> [128,1] so both packed halves get their k's coeff
        sc_ps = psS.tile([128, 1], F32, tag="smallps")
        nc.tensor.matmul(sc_ps[:], stackselT[:], scale64[:], start=True, stop=True)
        scale128 = work.tile([128, 1], F32, tag="scale128")
        nc.scalar.copy(scale128[:], sc_ps[:])
        bi_ps = psS.tile([128, 1], F32, tag="smallps")
        nc.tensor.matmul(bi_ps[:], stackselT[:], bias64[:], start=True, stop=True)
        bias128 = work.tile([128, 1], F32, tag="bias128")
        nc.scalar.copy(bias128[:], bi_ps[:])

        # ---- exp(BN(a)) in place, then transpose to n-major + normalize ---
        nc.scalar.activation(aT[:], aT[:], mybir.ActivationFunctionType.Exp,
                             bias=bias128[:], scale=scale128[:])

        soft = spool.tile([128, T, K], MM_DT)
        zsum = work.tile([128, T], F32, tag="zsum")
        for t in range(T):
            g = t // 4
            h = g % 2
            off = 512 * (g // 2) + 128 * (t % 4)
            sp = tpsum.tile([128, K], F32, tag="tp")
            ident_h = identity[0:K, 0:K] if h == 0 else ident_hi[64:128, :]
            nc.tensor.transpose(sp[:], aT[64 * h:64 * (h + 1), off:off + 128],
                                ident_h)
            nc.scalar.copy(soft[:, t, :], sp[:])
            nc.vector.reduce_sum(zsum[:, t:t + 1], sp[:], axis=mybir.AxisListType.X)
        zr = work.tile([128, T], F32, tag="zr")
        nc.vector.reciprocal(zr[:], zsum[:])
        for t in range(T):
            nc.vector.tensor_scalar_mul(soft[:, t, :], soft[:, t, :], zr[:, t:t + 1])

        # ---- pass 2: vlad^T = soft^T @ x, minus a_sum * clusters2^T -------
        for b in range(B_LOC):
            v_ps = psV.tile([K, 512], F32)
            s_ps = psS.tile([K, 1], F32, tag="smallps")
            for j in range(8):
                t = 8 * b + j
                nc.tensor.matmul(v_ps[:], soft[:, t, :], xnat[:, t, :],
                                 start=(j == 0), stop=(j == 7))
                # tiny free-dim-1 matmul: fp32r ISA forbids it, run it as fp32
                nc.tensor.matmul(s_ps[:], soft[:, t, :].bitcast(F32), ones_col[:],
                                 start=(j == 0), stop=(j == 7))
            asum = work.tile([K, 1], F32, tag="asum")
            nc.scalar.copy(asum[:], s_ps[:])
            corr = work.tile([K, D], F32, tag="corr")
            nc.vector.tensor_scalar_mul(corr[:], c2T[:], asum[:])
            vfin = work.tile([K, D], F32, tag="vfin")
            nc.vector.tensor_sub(vfin[:], v_ps[:], corr[:])
            # L2 norm over all D*K elements of this batch
            scr = work.tile([K, D], F32, tag="scr")
            sq = work.tile([K, 1], F32, tag="sq")
            nc.scalar.activation(scr[:], vfin[:],
                                 mybir.ActivationFunctionType.Square,
                                 accum_out=sq[:])
            n_ps = psS.tile([1, 1], F32, tag="smallps")
            nc.tensor.matmul(n_ps[:], ones_col[0:K, :], sq[:], start=True, stop=True)
            nrm = work.tile([1, 1], F32, tag="nrm")
            nc.scalar.activation(nrm[:], n_ps[:], mybir.ActivationFunctionType.Sqrt)
            nc.vector.tensor_scalar_max(nrm[:], nrm[:], L2_EPS)
            nc.vector.reciprocal(nrm[:], nrm[:])
            b_ps = psS.tile([K, 1], F32, tag="smallps")
            nc.tensor.matmul(b_ps[:], ones_row[:], nrm[:], start=True, stop=True)
            invn = work.tile([K, 1], F32, tag="invn")
            nc.scalar.copy(invn[:], b_ps[:])
            nc.vector.tensor_scalar_mul(vfin[:], vfin[:], invn[:])
            # transpose [K, D] -> [128, DCH, K] and write out
            vout = work.tile([128, DCH, K], F32, tag="vout")
            for c in range(DCH):
                fp = tpsum.tile([128, K], F32, tag="tp")
                nc.tensor.transpose(fp[:], vfin[:, 128 * c:128 * (c + 1)],
                                    identity[0:K, 0:K])
                eng_v = (c % 2 == 0)
                if eng_v:
                    nc.vector.tensor_copy(vout[:, c, :], fp[:])
                else:
                    nc.scalar.copy(vout[:, c, :], fp[:])
            nc.sync.dma_start(
                out_d.ap().rearrange("b (c p k) -> b p c k", p=128, k=K)[b],
                vout[:])

    nc.compile()
    return nc


def _get_nc():
    key = (MM_DT,)
    if key not in _cached:
        _cached[key] = build_kernel()
    return _cached[key]


def kernel(x=None, clusters=None, clusters2=None, gamma=None, beta=None, **kw):
    # Fall back to the deterministic setup_inputs() values for any input the
    # harness does not supply (they are fixed-seed constants of the problem).
    if clusters is None or clusters2 is None or gamma is None or beta is None:
        import jax
        cpu = jax.devices("cpu")[0]
        with jax.default_device(cpu):
            key = jax.random.key(0)
            k_x, k_c, k_c2 = jax.random.split(key, 3)
            init_sc = 1.0 / math.sqrt(D)
            if clusters is None:
                clusters = np.asarray(init_sc * jax.random.normal(k_c, (D, K)))
            if clusters2 is None:
                clusters2 = np.asarray(init_sc * jax.random.normal(k_c2, (1, D, K)))
            if gamma is None:
                gamma = np.ones((K,), np.float32)
            if beta is None:
                beta = np.zeros((K,), np.float32)
            if x is None:
                x = np.asarray(jax.random.normal(k_x, (B, N, D)))

    x = np.ascontiguousarray(np.asarray(x, dtype=np.float32))
    cl = np.ascontiguousarray(np.asarray(clusters, dtype=np.float32).reshape(D, K))
    c2 = np.ascontiguousarray(np.asarray(clusters2, dtype=np.float32).reshape(D, K))
    ga = np.ascontiguousarray(np.asarray(gamma, dtype=np.float32).reshape(K, 1))
    be = np.ascontiguousarray(np.asarray(beta, dtype=np.float32).reshape(K, 1))

    nc = _get_nc()
    in_maps = []
    for c in range(N_CORES):
        xs = np.ascontiguousarray(
            x[c * B_LOC:(c + 1) * B_LOC].reshape(R, D))
        in_maps.append({"x": xs, "clusters": cl, "clusters2": c2,
                        "gamma": ga, "beta": be})
    res = bass_utils.run_bass_kernel_spmd(
        nc, in_maps, core_ids=list(range(N_CORES)),
        **kw.get("_run_kwargs", {}))
    out = np.concatenate([res.results[c]["out"] for c in range(N_CORES)], axis=0)
    if kw.get("_return_results"):
        return out, res
    return out
